# revision 27
# baseline (speedup 1.0000x reference)
"""MultiHeadAttention kernel for Trainium2, 8-core head-parallel.

Problem: S=2048, B=2, D=1024, 16 heads of d=64 (batch_first=False).
Sharding: tensor-parallel over heads — each of the 8 cores computes 2 heads
(a 128-column slice of the output). No collectives: every core gets the full
(bf16, transposed) activations plus its own weight slice, computes its output
slice, and the host concatenates.

Per-core dataflow (layouts chosen so only V needs an on-device transpose,
done on the PE):
  q^T, k^T  [128=2*64 dout, T] = W_slice @ x^T          (PE, bf16, fp32 psum)
  v^T       [128, T] likewise, then PE-transposed to token-major v' [tok, 65]
            per head with a ones column appended (for softmax denominators)
  scores^T  [j, i] = k_h-tile . q_h-tile                (PE, K=64; the two
            heads' matmuls are emitted adjacently at partition bases 0/64 so
            they row-pack and run concurrently in the 128x128 array)
  attn^T    = exp(scores * 1/8)  [no max-subtract: scores ~ N(8, 1.7)]
                                                        (ScalarE, bf16 out)
  pv^T      [65, i] = v'^T . attn^T  — row 64 = softmax denominator,
            accumulated per j-tile right behind the exps (keeps PE dense)
  out^T     [64, i] = pv^T[0:64] * (1/pv^T[64])         (DVE; reciprocal on a
            [128, 8] reshape — single-partition reciprocal is ~6.5us on DVE)
Host gathers out^T [128, B*S] per core -> [S, B, 1024].
"""

import sys

if "/opt/trn_rl_repo" not in sys.path:
    sys.path.insert(0, "/opt/trn_rl_repo")

import numpy as np
import ml_dtypes

import concourse.bass as bass
import concourse.mybir as mybir
import concourse.tile as tile
from concourse import bacc

BF16 = mybir.dt.bfloat16
FP32 = mybir.dt.float32
NP_BF16 = ml_dtypes.bfloat16

D = 1024
NHEAD = 16
DH = 64
NCORES = 8
HPC = NHEAD // NCORES        # heads per core = 2
DC = HPC * DH                # per-core output dims = 128
KT = D // 128                # contraction tiles = 8
SCALE = 1.0 / float(np.sqrt(DH))


def build_program(S: int, B: int):
    """Build the single-core Bass program (identical across the 8 cores)."""
    assert S % 128 == 0
    T = S * B
    JT = S // 128                    # key tiles per (b, h)
    IC = min(1024, S)                # i-chunk (query positions per psum tile)
    assert S % IC == 0
    NIC = S // IC
    NI5 = IC // 512 if IC >= 512 else 1   # 512-wide matmuls per i-chunk
    MMW = IC // NI5                  # matmul free size (<=512)
    TB = 512 if S % 512 == 0 else S  # token block for projections
    TPB = S // TB                    # token blocks per batch
    VSUB = TB // 128                 # 128-token v tiles per block
    JTB = S // 128                   # v tiles per batch

    nc = bacc.Bacc(
        "TRN2", target_bir_lowering=False, debug=False, num_devices=NCORES
    )
    NTILE = T // TB
    # pre-tiled on host: tile (b*TPB+tb) is one contiguous [128, KT, TB] block
    xq = nc.dram_tensor("xq", [NTILE, 128, KT, TB], BF16, kind="ExternalInput")
    xk = nc.dram_tensor("xk", [NTILE, 128, KT, TB], BF16, kind="ExternalInput")
    xv = nc.dram_tensor("xv", [NTILE, 128, KT, TB], BF16, kind="ExternalInput")
    wq = nc.dram_tensor("wq", [D, DC], BF16, kind="ExternalInput")
    wk = nc.dram_tensor("wk", [D, DC], BF16, kind="ExternalInput")
    wv = nc.dram_tensor("wv", [D, DC], BF16, kind="ExternalInput")
    bqkv = nc.dram_tensor("bqkv", [DC, 3], FP32, kind="ExternalInput")
    ident = nc.dram_tensor("ident", [128, 128], BF16, kind="ExternalInput")
    out = nc.dram_tensor("out", [DC, T], FP32, kind="ExternalOutput")



    with tile.TileContext(nc) as tc:
        with (
            tc.tile_pool(name="const", bufs=1) as constp,
            tc.tile_pool(name="xin", bufs=1) as xinp,
            tc.tile_pool(name="qkv", bufs=1) as qkvp,
            tc.tile_pool(name="attn", bufs=3) as attnp,
            tc.tile_pool(name="vstg", bufs=2) as vstgp,
            tc.tile_pool(name="drain", bufs=1) as drainp,
            tc.tile_pool(name="outp", bufs=2) as outp,
            tc.tile_pool(name="ps", bufs=2, space="PSUM") as psp,
            tc.tile_pool(name="pv", bufs=2, space="PSUM") as pvp,
        ):
            wq_t = constp.tile([128, KT, DC], BF16, tag="wq")
            wk_t = constp.tile([128, KT, DC], BF16, tag="wk")
            wv_t = constp.tile([128, KT, DC], BF16, tag="wv")
            nc.sync.dma_start(out=wq_t[:], in_=wq[:, :].rearrange("(kt p) m -> p kt m", p=128))
            nc.sync.dma_start(out=wk_t[:], in_=wk[:, :].rearrange("(kt p) m -> p kt m", p=128))
            nc.sync.dma_start(out=wv_t[:], in_=wv[:, :].rearrange("(kt p) m -> p kt m", p=128))
            bqkv_t = constp.tile([DC, 3], FP32, tag="bqkv")
            ident_t = constp.tile([128, 128], BF16, tag="ident")

            q_b = []
            k_b = []
            v_b = []
            for b in range(B):
                q_b.append(qkvp.tile([128, S], BF16, tag=f"q{b}", name=f"q{b}"))
                k_b.append(qkvp.tile([128, S], BF16, tag=f"k{b}", name=f"k{b}"))
                v_b.append(
                    qkvp.tile([128, JTB, HPC, DH + 1], BF16, tag=f"v{b}", name=f"v{b}")
                )

            late_consts = [False]

            def emit_proj_qk(b, tb):
                t0 = b * S + tb * TB
                s0 = tb * TB
                tile_i = b * TPB + tb
                xq_t = xinp.tile([128, KT, TB], BF16, tag="xq", name="xq_t", bufs=2)
                xk_t = xinp.tile([128, KT, TB], BF16, tag="xk", name="xk_t", bufs=2)
                nc.gpsimd.dma_start(out=xq_t[:], in_=xq[tile_i, :, :, :])
                nc.gpsimd.dma_start(out=xk_t[:], in_=xk[tile_i, :, :, :])
                if not late_consts[0]:
                    # issued after the first x loads so they don't delay them
                    late_consts[0] = True
                    nc.sync.dma_start(out=bqkv_t[:], in_=bqkv[:, :])
                    nc.sync.dma_start(out=ident_t[:], in_=ident[:, :])

                ps_q = psp.tile([128, IC], FP32, tag="ps", name="ps_q")
                for kt in range(KT):
                    nc.tensor.matmul(
                        ps_q[:, :TB], wq_t[:, kt, :], xq_t[:, kt, :],
                        start=(kt == 0), stop=(kt == KT - 1),
                    )
                nc.vector.tensor_add(
                    q_b[b][:, s0 : s0 + TB],
                    ps_q[:, :TB],
                    bqkv_t[:, 0:1].to_broadcast((DC, TB)),
                )
                ps_k = psp.tile([128, IC], FP32, tag="ps", name="ps_k")
                for kt in range(KT):
                    nc.tensor.matmul(
                        ps_k[:, :TB], wk_t[:, kt, :], xk_t[:, kt, :],
                        start=(kt == 0), stop=(kt == KT - 1),
                    )
                nc.vector.tensor_add(
                    k_b[b][:, s0 : s0 + TB],
                    ps_k[:, :TB],
                    bqkv_t[:, 1:2].to_broadcast((DC, TB)),
                )

            def emit_proj_v(b, tb):
                # v: project to v^T like q/k (weight-stationary, wide N),
                # add bias, then PE-transpose 128x128 blocks to token-major.
                xv_t = xinp.tile([128, KT, TB], BF16, tag="xv", name="xv_t")
                nc.gpsimd.dma_start(out=xv_t[:], in_=xv[b * TPB + tb, :, :, :])
                ps_v = psp.tile([128, IC], FP32, tag="ps", name="ps_v")
                for kt in range(KT):
                    nc.tensor.matmul(
                        ps_v[:, :TB], wv_t[:, kt, :], xv_t[:, kt, :],
                        start=(kt == 0), stop=(kt == KT - 1),
                    )
                vT = vstgp.tile([128, TB], BF16, tag="vT", name="vT")
                nc.vector.tensor_add(
                    vT[:, :], ps_v[:, :TB], bqkv_t[:, 2:3].to_broadcast((DC, TB))
                )
                for sub in range(VSUB):
                    pt = psp.tile([128, 128], BF16, tag="ps", name="pt")
                    nc.tensor.transpose(
                        pt[:, :], vT[:, sub * 128 : (sub + 1) * 128], ident_t[:, :]
                    )
                    vt_idx = tb * VSUB + sub
                    nc.vector.tensor_copy(
                        v_b[b][:, vt_idx, :, 0:DH],
                        pt[:, :].rearrange("p (h d) -> p h d", h=HPC),
                    )

            def emit_attention(b, inject=None):
                for ic in range(NIC):
                    at = [
                        attnp.tile([128, JT, IC], BF16, tag="attn", name=f"at{hh}")
                        for hh in range(HPC)
                    ]
                    pv_ps = [
                        pvp.tile([128, IC], FP32, tag="pv", name=f"pv{hh}")
                        for hh in range(HPC)
                    ]
                    for jt in range(JT):
                        s_ps = [
                            psp.tile([128, IC], FP32, tag="ps", name=f"s{hh}")
                            for hh in range(HPC)
                        ]
                        # the two heads' matmuls sit at partition bases 0/64 ->
                        # row-packed, concurrent in the array
                        for n in range(NI5):
                            i0 = ic * IC + n * MMW
                            for hh in range(HPC):
                                p0 = hh * DH
                                nc.tensor.matmul(
                                    s_ps[hh][:, n * MMW : (n + 1) * MMW],
                                    k_b[b][p0 : p0 + DH, jt * 128 : (jt + 1) * 128],
                                    q_b[b][p0 : p0 + DH, i0 : i0 + MMW],
                                    start=True, stop=True,
                                )
                        for hh in range(HPC):
                            nc.scalar.activation(
                                out=at[hh][:, jt, :], in_=s_ps[hh][:, :],
                                func=mybir.ActivationFunctionType.Exp,
                                scale=SCALE,
                            )
                        # pv accumulation trails the exps by one j-tile
                        for hh in range(HPC):
                            for n in range(NI5):
                                nc.tensor.matmul(
                                    pv_ps[hh][0 : DH + 1, n * MMW : (n + 1) * MMW],
                                    v_b[b][:, jt, hh, :],
                                    at[hh][:, jt, n * MMW : (n + 1) * MMW],
                                    start=(jt == 0), stop=(jt == JT - 1),
                                )
                        if inject is not None:
                            inject(ic, jt)
                    for hh in range(HPC):
                        # Evacuate the full [65, IC] pv block to SBUF in one DVE
                        # copy so the psum slot frees immediately (next chunk's
                        # pv accumulation can start); the normalization chain
                        # then runs from SBUF off the critical path.
                        pvsb = outp.tile([DH + 1, IC], FP32, tag="pvsb", name="pvsb")
                        nc.vector.tensor_copy(pvsb[:, :], pv_ps[hh][0 : DH + 1, :])
                        # out = pv[0:64] / pv[64]: reshape the denominator row to
                        # [128, IC/128] for a fast reciprocal, broadcast it back
                        # over 64 partitions with a DMA, multiply on DVE.
                        rsh = drainp.tile([128, IC // 128], FP32, tag="rsh", name="rsh")
                        nc.sync.dma_start(out=rsh[:], in_=pvsb[DH : DH + 1, :])
                        rec = drainp.tile([128, IC // 128], FP32, tag="rec", name="rec")
                        nc.vector.reciprocal(rec[:], rsh[:])
                        rrow = drainp.tile([1, IC], FP32, tag="rrow", name="rrow")
                        nc.sync.dma_start(out=rrow[:], in_=rec[:])
                        denb = drainp.tile([DH, IC], FP32, tag="denb", name="denb")
                        rr_ap = rrow[0:1, :]
                        nc.sync.dma_start(
                            out=denb[:],
                            in_=bass.AP(
                                tensor=rr_ap.tensor,
                                offset=rr_ap.offset,
                                ap=[list(rr_ap.ap[0]), [0, DH], list(rr_ap.ap[1])],
                            ),
                        )
                        nc.vector.tensor_mul(pvsb[0:DH, :], pvsb[0:DH, :], denb[:])
                        nc.sync.dma_start(
                            out=out[
                                hh * DH : (hh + 1) * DH,
                                b * S + ic * IC : b * S + (ic + 1) * IC,
                            ],
                            in_=pvsb[0:DH, :],
                        )

            # batch 0 projections (q/k first so attention can start early)
            nc.vector.memset(v_b[0][:, :, :, DH : DH + 1], 1.0)
            for tb in range(TPB):
                emit_proj_qk(0, tb)
            for tb in range(TPB):
                emit_proj_v(0, tb)

            if B > 1:
                # batch 1 projection units are injected into batch 0's
                # attention emission so the PE fills ScalarE-paced gaps
                # instead of running a serial ACT-idle projection phase.
                units = [lambda: nc.vector.memset(v_b[1][:, :, :, DH : DH + 1], 1.0)]
                units += [
                    (lambda tb_: lambda: emit_proj_qk(1, tb_))(tb) for tb in range(TPB)
                ]
                units += [
                    (lambda tb_: lambda: emit_proj_v(1, tb_))(tb) for tb in range(TPB)
                ]
                n_units = len(units)
                n_points = NIC * JT
                ui = [0]

                def inject(ic, jt):
                    point = ic * JT + jt
                    want = (point + 1) * n_units // n_points
                    while ui[0] < want:
                        units[ui[0]]()
                        ui[0] += 1

                emit_attention(0, inject=inject)
                while ui[0] < n_units:
                    units[ui[0]]()
                    ui[0] += 1
                emit_attention(1)
            else:
                emit_attention(0)

    nc.finalize()
    return nc


_PROGRAM_CACHE = {}


def _get_program(S, B):
    key = (S, B)
    if key not in _PROGRAM_CACHE:
        _PROGRAM_CACHE[key] = build_program(S, B)
    return _PROGRAM_CACHE[key]


def make_in_maps(query, key, value, Wq, bq, Wk, bk, Wv, bv):
    S, B, D_ = query.shape
    assert D_ == D
    T = S * B

    TB = 512 if S % 512 == 0 else S
    NTILE = T // TB

    def xt(a):
        # [S, B, D] -> transposed [D, B*S] -> pre-tiled [NTILE, 128, KT, TB]
        # bf16 so each SBUF tile is one contiguous 1MB DMA read.
        aT = np.asarray(a, np.float32).transpose(2, 1, 0).reshape(D_, T)
        a4 = aT.reshape(KT, 128, NTILE, TB).transpose(2, 1, 0, 3)
        return np.ascontiguousarray(a4).astype(NP_BF16)

    xqh, xkh, xvh = xt(query), xt(key), xt(value)
    identh = np.eye(128, dtype=NP_BF16)
    in_maps = []
    for c in range(NCORES):
        rows = slice(c * DC, (c + 1) * DC)
        in_maps.append(
            {
                "xq": xqh, "xk": xkh, "xv": xvh,
                "wq": np.ascontiguousarray(np.asarray(Wq)[rows, :].T).astype(NP_BF16),
                "wk": np.ascontiguousarray(np.asarray(Wk)[rows, :].T).astype(NP_BF16),
                "wv": np.ascontiguousarray(np.asarray(Wv)[rows, :].T).astype(NP_BF16),
                "bqkv": np.ascontiguousarray(
                    np.stack(
                        [np.asarray(bq)[rows], np.asarray(bk)[rows], np.asarray(bv)[rows]],
                        axis=1,
                    )
                ).astype(np.float32),
                "ident": identh,
            }
        )
    return in_maps


def gather_output(results, S, B):
    full = np.empty((S, B, D), np.float32)
    for c in range(NCORES):
        o = np.asarray(results[c]["out"], np.float32)  # [DC, B*S]
        full[:, :, c * DC : (c + 1) * DC] = o.reshape(DC, B, S).transpose(2, 1, 0)
    return full


def kernel(query, key, value, Wq, bq, Wk, bk, Wv, bv):
    from concourse.bass_utils import run_bass_kernel_spmd

    S, B, _ = query.shape
    nc = _get_program(S, B)
    in_maps = make_in_maps(query, key, value, Wq, bq, Wk, bk, Wv, bv)
    res = run_bass_kernel_spmd(nc, in_maps, list(range(NCORES)))
    return gather_output(res.results, S, B)


# revision 32
# speedup vs baseline: 1.2853x; 1.2853x over previous
"""MultiHeadAttention kernel for Trainium2, 8-core head-parallel.

Problem: S=2048, B=2, D=1024, 16 heads of d=64 (batch_first=False).
Sharding: tensor-parallel over heads — each of the 8 cores computes 2 heads
(a 128-column slice of the output). No collectives: every core gets the full
(bf16, transposed) activations plus its own weight slice, computes its output
slice, and the host concatenates.

Per-core dataflow (layouts chosen so only V needs an on-device transpose,
done on the PE):
  q^T, k^T  [128=2*64 dout, T] = W_slice @ x^T          (PE, bf16, fp32 psum)
  v^T       [128, T] likewise, then PE-transposed to token-major v' [tok, 65]
            per head with a ones column appended (for softmax denominators)
  scores^T  [j, i] = k_h-tile . q_h-tile                (PE, K=64; the two
            heads' matmuls are emitted adjacently at partition bases 0/64 so
            they row-pack and run concurrently in the 128x128 array)
  attn^T    = exp(scores * 1/8)  [no max-subtract: scores ~ N(8, 1.7)]
                                                        (ScalarE, bf16 out)
  pv^T      [65, i] = v'^T . attn^T  — row 64 = softmax denominator,
            accumulated per j-tile right behind the exps (keeps PE dense)
  out^T     [64, i] = pv^T[0:64] * (1/pv^T[64])         (DVE; reciprocal on a
            [128, 8] reshape — single-partition reciprocal is ~6.5us on DVE)
Host gathers out^T [128, B*S] per core -> [S, B, 1024].
"""

import sys

if "/opt/trn_rl_repo" not in sys.path:
    sys.path.insert(0, "/opt/trn_rl_repo")

import numpy as np
import ml_dtypes

import concourse.bass as bass
import concourse.mybir as mybir
import concourse.tile as tile
from concourse import bacc

BF16 = mybir.dt.bfloat16
FP32 = mybir.dt.float32
NP_BF16 = ml_dtypes.bfloat16

D = 1024
NHEAD = 16
DH = 64
NCORES = 8
HPC = NHEAD // NCORES        # heads per core = 2
DC = HPC * DH                # per-core output dims = 128
KT = D // 128                # contraction tiles = 8
SCALE = 1.0 / float(np.sqrt(DH))


def build_program(S: int, B: int):
    """Build the single-core Bass program (identical across the 8 cores)."""
    assert S % 128 == 0
    T = S * B
    JT = S // 128                    # key tiles per (b, h)
    IC = min(1024, S)                # i-chunk (query positions per psum tile)
    assert S % IC == 0
    NIC = S // IC
    NI5 = IC // 512 if IC >= 512 else 1   # 512-wide matmuls per i-chunk
    MMW = IC // NI5                  # matmul free size (<=512)
    TB = 512 if S % 512 == 0 else S  # token block for projections
    TPB = S // TB                    # token blocks per batch
    VSUB = TB // 128                 # 128-token v tiles per block
    JTB = S // 128                   # v tiles per batch

    nc = bacc.Bacc(
        "TRN2", target_bir_lowering=False, debug=False, num_devices=NCORES
    )
    NTILE = T // TB
    # pre-tiled on host: tile (b*TPB+tb) is one contiguous [128, KT, TB] block
    xq = nc.dram_tensor("xq", [NTILE, 128, KT, TB], BF16, kind="ExternalInput")
    xk = nc.dram_tensor("xk", [NTILE, 128, KT, TB], BF16, kind="ExternalInput")
    xv = nc.dram_tensor("xv", [NTILE, 128, KT, TB], BF16, kind="ExternalInput")
    wq = nc.dram_tensor("wq", [D, DC], BF16, kind="ExternalInput")
    wk = nc.dram_tensor("wk", [D, DC], BF16, kind="ExternalInput")
    wv = nc.dram_tensor("wv", [D, DC], BF16, kind="ExternalInput")
    bqkv = nc.dram_tensor("bqkv", [DC, 3], FP32, kind="ExternalInput")
    ident = nc.dram_tensor("ident", [128, 128], BF16, kind="ExternalInput")
    out = nc.dram_tensor("out", [DC, T], FP32, kind="ExternalOutput")



    with tile.TileContext(nc) as tc:
        with (
            tc.tile_pool(name="const", bufs=1) as constp,
            tc.tile_pool(name="xin", bufs=1) as xinp,
            tc.tile_pool(name="qkv", bufs=1) as qkvp,
            tc.tile_pool(name="attn", bufs=3) as attnp,
            tc.tile_pool(name="vstg", bufs=2) as vstgp,
            tc.tile_pool(name="drain", bufs=1) as drainp,
            tc.tile_pool(name="outp", bufs=2) as outp,
            tc.tile_pool(name="ps", bufs=2, space="PSUM") as psp,
            tc.tile_pool(name="pv", bufs=2, space="PSUM") as pvp,
        ):
            wq_t = constp.tile([128, KT, DC], BF16, tag="wq")
            wk_t = constp.tile([128, KT, DC], BF16, tag="wk")
            wv_t = constp.tile([128, KT, DC], BF16, tag="wv")
            nc.sync.dma_start(out=wq_t[:], in_=wq[:, :].rearrange("(kt p) m -> p kt m", p=128))
            nc.sync.dma_start(out=wk_t[:], in_=wk[:, :].rearrange("(kt p) m -> p kt m", p=128))
            nc.sync.dma_start(out=wv_t[:], in_=wv[:, :].rearrange("(kt p) m -> p kt m", p=128))
            bqkv_t = constp.tile([DC, 3], FP32, tag="bqkv")
            ident_t = constp.tile([128, 128], BF16, tag="ident")

            q_b = []
            k_b = []
            v_b = []
            for b in range(B):
                q_b.append(qkvp.tile([128, S], BF16, tag=f"q{b}", name=f"q{b}"))
                k_b.append(qkvp.tile([128, S], BF16, tag=f"k{b}", name=f"k{b}"))
                v_b.append(
                    qkvp.tile([128, JTB, HPC, DH + 1], BF16, tag=f"v{b}", name=f"v{b}")
                )

            late_consts = [False]

            def emit_proj_qk(b, tb):
                t0 = b * S + tb * TB
                s0 = tb * TB
                tile_i = b * TPB + tb
                xq_t = xinp.tile([128, KT, TB], BF16, tag="xq", name="xq_t", bufs=2)
                xk_t = xinp.tile([128, KT, TB], BF16, tag="xk", name="xk_t", bufs=2)
                nc.gpsimd.dma_start(out=xq_t[:], in_=xq[tile_i, :, :, :])
                nc.gpsimd.dma_start(out=xk_t[:], in_=xk[tile_i, :, :, :])
                if not late_consts[0]:
                    # issued after the first x loads so they don't delay them
                    late_consts[0] = True
                    nc.sync.dma_start(out=bqkv_t[:], in_=bqkv[:, :])
                    nc.sync.dma_start(out=ident_t[:], in_=ident[:, :])

                ps_q = psp.tile([128, IC], FP32, tag="ps", name="ps_q")
                for kt in range(KT):
                    nc.tensor.matmul(
                        ps_q[:, :TB], wq_t[:, kt, :], xq_t[:, kt, :],
                        start=(kt == 0), stop=(kt == KT - 1),
                    )
                nc.vector.tensor_add(
                    q_b[b][:, s0 : s0 + TB],
                    ps_q[:, :TB],
                    bqkv_t[:, 0:1].to_broadcast((DC, TB)),
                )
                ps_k = psp.tile([128, IC], FP32, tag="ps", name="ps_k")
                for kt in range(KT):
                    nc.tensor.matmul(
                        ps_k[:, :TB], wk_t[:, kt, :], xk_t[:, kt, :],
                        start=(kt == 0), stop=(kt == KT - 1),
                    )
                nc.vector.tensor_add(
                    k_b[b][:, s0 : s0 + TB],
                    ps_k[:, :TB],
                    bqkv_t[:, 1:2].to_broadcast((DC, TB)),
                )

            def emit_proj_v(b, tb):
                # v: project to v^T like q/k (weight-stationary, wide N),
                # add bias, then PE-transpose 128x128 blocks to token-major.
                xv_t = xinp.tile([128, KT, TB], BF16, tag="xv", name="xv_t")
                nc.gpsimd.dma_start(out=xv_t[:], in_=xv[b * TPB + tb, :, :, :])
                ps_v = psp.tile([128, IC], FP32, tag="ps", name="ps_v")
                for kt in range(KT):
                    nc.tensor.matmul(
                        ps_v[:, :TB], wv_t[:, kt, :], xv_t[:, kt, :],
                        start=(kt == 0), stop=(kt == KT - 1),
                    )
                vT = vstgp.tile([128, TB], BF16, tag="vT", name="vT")
                nc.vector.tensor_add(
                    vT[:, :], ps_v[:, :TB], bqkv_t[:, 2:3].to_broadcast((DC, TB))
                )
                for sub in range(VSUB):
                    pt = psp.tile([128, 128], BF16, tag="ps", name="pt")
                    nc.tensor.transpose(
                        pt[:, :], vT[:, sub * 128 : (sub + 1) * 128], ident_t[:, :]
                    )
                    vt_idx = tb * VSUB + sub
                    nc.vector.tensor_copy(
                        v_b[b][:, vt_idx, :, 0:DH],
                        pt[:, :].rearrange("p (h d) -> p h d", h=HPC),
                    )

            def emit_attention(b, inject=None):
                for ic in range(NIC):
                    at = [
                        attnp.tile([128, JT, IC], BF16, tag="attn", name=f"at{hh}")
                        for hh in range(HPC)
                    ]
                    pv_ps = [
                        pvp.tile([128, IC], FP32, tag="pv", name=f"pv{hh}")
                        for hh in range(HPC)
                    ]
                    for jt in range(JT):
                        if inject is not None:
                            # must run before this jt's consumers are emitted:
                            # program order defines the data each read sees
                            inject(ic, jt)
                        s_ps = [
                            psp.tile([128, IC], FP32, tag="ps", name=f"s{hh}")
                            for hh in range(HPC)
                        ]
                        # same-weights matmuls adjacent (they stream at N cycles
                        # with the LDW overlapped); the two heads still overlap
                        # via distinct row groups (partition bases 0/64)
                        for hh in range(HPC):
                            p0 = hh * DH
                            for n in range(NI5):
                                i0 = ic * IC + n * MMW
                                nc.tensor.matmul(
                                    s_ps[hh][:, n * MMW : (n + 1) * MMW],
                                    k_b[b][p0 : p0 + DH, jt * 128 : (jt + 1) * 128],
                                    q_b[b][p0 : p0 + DH, i0 : i0 + MMW],
                                    start=True, stop=True,
                                )
                        for hh in range(HPC):
                            nc.scalar.activation(
                                out=at[hh][:, jt, :], in_=s_ps[hh][:, :],
                                func=mybir.ActivationFunctionType.Exp,
                                scale=SCALE,
                            )
                        # pv accumulation trails the exps by one j-tile
                        for hh in range(HPC):
                            for n in range(NI5):
                                nc.tensor.matmul(
                                    pv_ps[hh][0 : DH + 1, n * MMW : (n + 1) * MMW],
                                    v_b[b][:, jt, hh, :],
                                    at[hh][:, jt, n * MMW : (n + 1) * MMW],
                                    start=(jt == 0), stop=(jt == JT - 1),
                                )
                    for hh in range(HPC):
                        # Evacuate the full [65, IC] pv block to SBUF in one DVE
                        # copy so the psum slot frees immediately (next chunk's
                        # pv accumulation can start); the normalization chain
                        # then runs from SBUF off the critical path.
                        pvsb = outp.tile([DH + 1, IC], FP32, tag="pvsb", name="pvsb")
                        nc.vector.tensor_copy(pvsb[:, :], pv_ps[hh][0 : DH + 1, :])
                        # out = pv[0:64] / pv[64]: reshape the denominator row to
                        # [128, IC/128] for a fast reciprocal, broadcast it back
                        # over 64 partitions with a DMA, multiply on DVE.
                        rsh = drainp.tile([128, IC // 128], FP32, tag="rsh", name="rsh")
                        nc.sync.dma_start(out=rsh[:], in_=pvsb[DH : DH + 1, :])
                        rec = drainp.tile([128, IC // 128], FP32, tag="rec", name="rec")
                        nc.vector.reciprocal(rec[:], rsh[:])
                        rrow = drainp.tile([1, IC], FP32, tag="rrow", name="rrow")
                        nc.sync.dma_start(out=rrow[:], in_=rec[:])
                        denb = drainp.tile([DH, IC], FP32, tag="denb", name="denb")
                        rr_ap = rrow[0:1, :]
                        nc.sync.dma_start(
                            out=denb[:],
                            in_=bass.AP(
                                tensor=rr_ap.tensor,
                                offset=rr_ap.offset,
                                ap=[list(rr_ap.ap[0]), [0, DH], list(rr_ap.ap[1])],
                            ),
                        )
                        nc.vector.tensor_mul(pvsb[0:DH, :], pvsb[0:DH, :], denb[:])
                        nc.sync.dma_start(
                            out=out[
                                hh * DH : (hh + 1) * DH,
                                b * S + ic * IC : b * S + (ic + 1) * IC,
                            ],
                            in_=pvsb[0:DH, :],
                        )

            # Prologue: just enough projection for batch-0 attention to start
            # (scores for chunk 0 need q columns 0:IC and the k tiles as the
            # j-loop reaches them). Everything else — remaining b0 projection
            # units and all of b1's — is injected into the attention emission
            # at j-tile milestones, so the kernel is one continuous pipeline
            # and the PE never sits in a long ACT-idle projection phase.
            nc.vector.memset(v_b[0][:, :, :, DH : DH + 1], 1.0)
            emit_proj_qk(0, 0)
            if TPB > 1:
                emit_proj_qk(0, 1)

            units = [(lambda tb_=tb: emit_proj_v(0, tb_)) for tb in range(TPB)]
            # interleave remaining b0 qk with early v units: v(0), qk(2), v(1),
            # qk(3), v(2), v(3) for TPB=4
            for i, tb in enumerate(range(2, TPB)):
                units.insert(2 * i + 1, (lambda tb_=tb: emit_proj_qk(0, tb_)))
            n_b0 = len(units)
            if B > 1:
                units.append(lambda: nc.vector.memset(v_b[1][:, :, :, DH : DH + 1], 1.0))
                units += [(lambda tb_=tb: emit_proj_qk(1, tb_)) for tb in range(TPB)]
                units += [(lambda tb_=tb: emit_proj_v(1, tb_)) for tb in range(TPB)]

            n_points = NIC * JT
            # b0 units fire one-per-point from point 0 (k tiles are consumed by
            # the j-loop at 4 tiles/tb, v tiles trail the pv matmuls slightly);
            # b1 units spread over the remaining points of b0's attention.
            fire = [2 * i for i in range(n_b0)]
            n_rest = len(units) - n_b0
            base = 2 * n_b0
            for i in range(n_rest):
                fire.append(base + (i * max(n_points - base, 1)) // max(n_rest, 1))
            ui = [0]

            def inject(ic, jt):
                point = ic * JT + jt
                while ui[0] < len(units) and fire[ui[0]] <= point:
                    units[ui[0]]()
                    ui[0] += 1

            emit_attention(0, inject=inject)
            while ui[0] < len(units):
                units[ui[0]]()
                ui[0] += 1
            if B > 1:
                emit_attention(1)

    nc.finalize()
    return nc


_PROGRAM_CACHE = {}


def _get_program(S, B):
    key = (S, B)
    if key not in _PROGRAM_CACHE:
        _PROGRAM_CACHE[key] = build_program(S, B)
    return _PROGRAM_CACHE[key]


def make_in_maps(query, key, value, Wq, bq, Wk, bk, Wv, bv):
    S, B, D_ = query.shape
    assert D_ == D
    T = S * B

    TB = 512 if S % 512 == 0 else S
    NTILE = T // TB

    def xt(a):
        # [S, B, D] -> transposed [D, B*S] -> pre-tiled [NTILE, 128, KT, TB]
        # bf16 so each SBUF tile is one contiguous 1MB DMA read.
        aT = np.asarray(a, np.float32).transpose(2, 1, 0).reshape(D_, T)
        a4 = aT.reshape(KT, 128, NTILE, TB).transpose(2, 1, 0, 3)
        return np.ascontiguousarray(a4).astype(NP_BF16)

    xqh, xkh, xvh = xt(query), xt(key), xt(value)
    identh = np.eye(128, dtype=NP_BF16)
    in_maps = []
    for c in range(NCORES):
        rows = slice(c * DC, (c + 1) * DC)
        in_maps.append(
            {
                "xq": xqh, "xk": xkh, "xv": xvh,
                "wq": np.ascontiguousarray(np.asarray(Wq)[rows, :].T).astype(NP_BF16),
                "wk": np.ascontiguousarray(np.asarray(Wk)[rows, :].T).astype(NP_BF16),
                "wv": np.ascontiguousarray(np.asarray(Wv)[rows, :].T).astype(NP_BF16),
                "bqkv": np.ascontiguousarray(
                    np.stack(
                        [np.asarray(bq)[rows], np.asarray(bk)[rows], np.asarray(bv)[rows]],
                        axis=1,
                    )
                ).astype(np.float32),
                "ident": identh,
            }
        )
    return in_maps


def gather_output(results, S, B):
    full = np.empty((S, B, D), np.float32)
    for c in range(NCORES):
        o = np.asarray(results[c]["out"], np.float32)  # [DC, B*S]
        full[:, :, c * DC : (c + 1) * DC] = o.reshape(DC, B, S).transpose(2, 1, 0)
    return full


def kernel(query, key, value, Wq, bq, Wk, bk, Wv, bv):
    from concourse.bass_utils import run_bass_kernel_spmd

    S, B, _ = query.shape
    nc = _get_program(S, B)
    in_maps = make_in_maps(query, key, value, Wq, bq, Wk, bk, Wv, bv)
    res = run_bass_kernel_spmd(nc, in_maps, list(range(NCORES)))
    return gather_output(res.results, S, B)
